# revision 53
# baseline (speedup 1.0000x reference)
"""Trainium2 Bass kernel for nn_FFReModel (2-layer GPT-2 + tied LM head).

Sharding: 8 cores = 4 batches x 2 token chunks. The pair of cores owning a
batch splits its 1024-token sequence in half (core A: tokens 0-511, core B:
512-1023). Each core runs the transformer only for its own 512 tokens; the
per-layer K/V needed for attention are exchanged between the pair with a
DRAM AllGather (causality means chunk 0 never attends to chunk 1, so the
exchange is all either core needs). Each core then computes the LM head for
its own 512 tokens over the FULL vocab and writes bf16 logits; the host
stitches chunks and upcasts. This removes the redundant transformer compute
of the old batch x vocab-half sharding (~25% of per-core tensor work) and
halves the logits store traffic.

Layout: activations are feature-major ("xT": [D partitions, T free]) so every
linear is matmul(out[dout, t], lhsT=W[din, dout], rhs=xT[din, t]) accumulated
over din tiles. Attention scores are computed key-major ([tk, tq]); softmax
uses no max-subtraction (scores are bounded: 0.02-scale random weights),
sum-of-exp comes free from an appended ones-column in the V operand, and the
per-query normalization is broadcast across partitions with a rank-1 (K=1)
matmul of the reciprocal row. Attention runs over LOCAL key slots (own chunk
first from SBUF, then the peer chunk gathered from the exchange buffer via
host-supplied row indices) so one SPMD program serves both chunk roles; the
own-chunk causal mask is a static triangle, peer-chunk and validity masks are
host-provided per-slot exp-bias columns.
"""
import numpy as np
import ml_dtypes
from contextlib import ExitStack

import concourse.bass as bass
import concourse.tile as tile
from concourse import bacc, mybir
from concourse.bass_utils import run_bass_kernel_spmd
from concourse.masks import make_identity

BF = mybir.dt.bfloat16
F32 = mybir.dt.float32
I32 = mybir.dt.int32
AF = mybir.ActivationFunctionType
OP = mybir.AluOpType

B, L, V, D, H, DH, NL, F = 4, 1024, 50257, 768, 12, 64, 2, 3072
CH = 512               # tokens owned per core (one chunk)
TO = CH // 128         # 4 own token tiles
KT = D // 128          # 6 feature k-tiles
FT = F // 128          # 24 mlp feature tiles
NSLOT = L // 128       # 8 global key slots
VPAD = 50688           # padded vocab (99 tiles of 512)
NVT = VPAD // 512      # 99
KVW = KT * 512 + TO * 768   # 6144 packed kv columns (k: 6x512, v: 4x768)
PAIRS = [[0, 1], [2, 3], [4, 5], [6, 7]]
NEGBIG = -1e9
GELU_MODE = "hw"

# packed f32 param column offsets (all [128, x])
_PC = {}
_off = 0
for _n, _c in [("llb", KT), ("lnfg", KT), ("lnfb", KT),
               ("ln1g", NL * KT), ("ln1b", NL * KT),
               ("ln2g", NL * KT), ("ln2b", NL * KT),
               ("bqk", NL * 12), ("bo", NL * KT),
               ("bfc", NL * FT), ("bpr", NL * KT),
               ("valid", TO), ("vkeep", TO), ("pbias", TO)]:
    _PC[_n] = (_off, _c)
    _off += _c
PCOLS = _off


def _emit(nc, flags):
    """Emit the whole per-core program into nc (inside a TileContext)."""
    NOWN = min(TO, flags["maxkt"])   # own-chunk key slots any core needs
    NPEER = min(TO, flags["maxkt"])  # peer-chunk key slots any core needs
    NTOT = NOWN + NPEER
    # ---- DRAM I/O ----
    d_h0 = nc.dram_tensor("h0", [128, KT * CH], F32, kind="ExternalInput").ap()
    d_prow = nc.dram_tensor("prow", [128, 1], I32, kind="ExternalInput").ap()
    d_par = nc.dram_tensor("par", [128, PCOLS], F32, kind="ExternalInput").ap()
    d_lmw = nc.dram_tensor("lmw", [NVT, 128, KT * 512], BF, kind="ExternalInput").ap()
    d_wqk = nc.dram_tensor("wqk", [NL, D, 1536], BF, kind="ExternalInput").ap()
    d_wv = nc.dram_tensor("wv", [NL, D, D], BF, kind="ExternalInput").ap()
    d_bv = nc.dram_tensor("bv", [NL, D], BF, kind="ExternalInput").ap()
    d_wo = nc.dram_tensor("wo", [NL, D, D], BF, kind="ExternalInput").ap()
    d_wfc = nc.dram_tensor("wfc", [NL, 128, FT * KT * 128], BF, kind="ExternalInput").ap()
    d_wpr = nc.dram_tensor("wpr", [NL, 128, FT * KT * 128], BF, kind="ExternalInput").ap()
    d_out = nc.dram_tensor("out", [CH, VPAD], BF, kind="ExternalOutput").ap()
    # k/v exchange buffers (per layer): local contribution and pair allgather.
    # Split k from v so the k exchange launches before v/q are even computed.
    d_kloc = [[nc.dram_tensor(f"kloc{l}_{hf}", [128, 3 * 512], BF,
                              kind="Internal").ap() for hf in range(2)]
              for l in range(NL)]
    d_kag = [[nc.dram_tensor(f"kag{l}_{hf}", [256, 3 * 512], BF,
                             kind="Internal").ap() for hf in range(2)]
             for l in range(NL)]
    d_vloc = [nc.dram_tensor(f"vloc{l}", [128, TO * 768], BF, kind="Internal").ap()
              for l in range(NL)]
    d_vag = [nc.dram_tensor(f"vag{l}", [256, TO * 768], BF, kind="Internal").ap()
             for l in range(NL)]

    tc = nc._tc  # set by caller
    ctx = nc._ctx

    # ---- persistent pools ----
    cst = ctx.enter_context(tc.tile_pool(name="cst", bufs=1))
    hp = ctx.enter_context(tc.tile_pool(name="hp", bufs=1))
    act = ctx.enter_context(tc.tile_pool(name="act", bufs=1))

    # constants / params
    prow_sb = cst.tile([128, 1], I32)
    nc.sync.dma_start(prow_sb[:], d_prow[:])
    par = cst.tile([128, PCOLS], F32)
    nc.sync.dma_start(par[:], d_par[:])

    def P(name, i=0):
        o, n = _PC[name]
        return par[:, o + i:o + i + 1]

    def PL(name, l, i):
        o, n = _PC[name]
        per = n // NL
        return par[:, o + l * per + i:o + l * per + i + 1]

    ones_row = cst.tile([1, 128], BF)
    nc.vector.memset(ones_row[:], 1.0)
    # full 128x128 all-ones stationary: partition sums land pre-broadcast and
    # stay full-height (narrow stationaries poison the PE into half-height
    # mode for ~24us around every use)
    ones128 = cst.tile([128, 128], BF)
    nc.vector.memset(ones128[:], 1.0)
    ones128f = cst.tile([128, 128], F32)
    nc.vector.memset(ones128f[:], 1.0)
    eps_col = cst.tile([128, 1], F32)
    nc.vector.memset(eps_col[:], 1e-5)
    # per-own-slot causal keep-masks (trim[s][p, x + 512*half] = 1 iff
    # x >= p + 128s), duplicated for the two heads sharing one merged
    # [128, 1024] exp tile
    trim = [cst.tile([128, 2 * CH], BF, tag=f"trim{s}", name=f"trim{s}")
            for s in range(TO)]
    for s in range(TO):
        nc.gpsimd.memset(trim[s][:], 0.0)
        for half in range(2):
            nc.gpsimd.affine_select(
                out=trim[s][:, half * 512:(half + 1) * 512],
                in_=trim[s][:, half * 512:(half + 1) * 512],
                compare_op=OP.is_gt, fill=1.0,
                base=128 * s, pattern=[[-1, 512]], channel_multiplier=1)

    bv_sb = [cst.tile([1, D], BF, tag=f"bv{l}", name=f"bv{l}") for l in range(NL)]
    for l in range(NL):
        nc.sync.dma_start(bv_sb[l][:], d_bv[l:l + 1, :])

    # residual stream, fp32 feature-major
    h = [hp.tile([128, CH], F32, tag=f"h{k}", name=f"h{k}") for k in range(KT)]
    # v token-major per LOCAL slot (0..3 own chunk, 4..7 peer chunk),
    # 128-stride per head: col 0 = ones (sumexp), cols 64..127 = v
    v_tok = [hp.tile([128, H * 128], BF, tag=f"vtok{s}", name=f"vtok{s}")
             for s in range(NTOT)]
    for s in range(NTOT):
        nc.gpsimd.memset(v_tok[s][:], 1.0)
    # zero-padded q score operands: head-even in rows 0..63 / cols 0..511,
    # head-odd in rows 64..127 / cols 512..1023. The zero quadrants (written
    # once, never touched again) let score matmuls contract over the full 128
    # partitions so the PE never drops into half-height mode.
    qpad = [hp.tile([128, 2 * CH], BF, tag=f"qpad{m}", name=f"qpad{m}")
            for m in range(6)]
    for m in range(6):
        nc.gpsimd.memset(qpad[m][:], 0.0)

    # ---------- layernorm helper ----------
    def layernorm(tag, src_tiles, g_col, b_col, skip_bias, dst_tiles,
                  tmajor=False):
        """dst = LN(src) * g + b, feature-major, bf16 out.

        Partition sums use the all-ones [128,128] stationary, so the result
        lands pre-broadcast across partitions and the PE stays in full-height
        mode. tmajor=True writes dst in token-tile-major order so consumers
        keyed on token tiles (the LM head) can start on tile 0 early.
        """
        with tc.tile_pool(name=f"{tag}_sb", bufs=1) as lp, \
             tc.tile_pool(name=f"{tag}_ps", bufs=2, space="PSUM") as pp:
            xbs = []
            for k in range(KT):
                xb = lp.tile([128, CH], BF, tag=f"xb{k}")
                nc.vector.tensor_copy(xb[:], src_tiles[k][:])
                xbs.append(xb)
            r_sx = pp.tile([128, CH], F32, tag="r_sx")
            r_sx2 = pp.tile([128, CH], F32, tag="r_sx2")
            for k in range(KT):
                nc.tensor.matmul(r_sx[:], ones128[:], xbs[k][:],
                                 start=(k == 0), stop=(k == KT - 1))
            for k in range(KT):
                sq = lp.tile([128, CH], BF, tag="sq", bufs=2)
                nc.vector.tensor_tensor(sq[:], xbs[k][:], xbs[k][:], op=OP.mult)
                nc.tensor.matmul(r_sx2[:], ones128[:], sq[:],
                                 start=(k == 0), stop=(k == KT - 1))
            m_bc = lp.tile([128, CH], F32, tag="m_bc")
            nc.vector.tensor_scalar_mul(m_bc[:], r_sx[:], 1.0 / D)
            m2 = lp.tile([128, CH], F32, tag="m2")
            nc.scalar.activation(m2[:], r_sx[:], AF.Square, scale=1.0 / D)
            var = lp.tile([128, CH], F32, tag="var")
            nc.vector.scalar_tensor_tensor(
                out=var[:], in0=r_sx2[:], scalar=1.0 / D, in1=m2[:],
                op0=OP.mult, op1=OP.subtract)
            sd = lp.tile([128, CH], F32, tag="sd")
            nc.scalar.activation(sd[:], var[:], AF.Sqrt, bias=eps_col[:])
            r_bc = lp.tile([128, CH], F32, tag="r_bc")
            nc.vector.reciprocal_approx_fast(out=r_bc[:], in_=sd[:])
            tslices = ([slice(t * 128, (t + 1) * 128) for t in range(TO)]
                       if tmajor else [slice(0, CH)])
            for ts in tslices:
                for k in range(KT):
                    t1 = lp.tile([128, CH], BF, tag="t1", bufs=2)
                    nc.vector.tensor_tensor(t1[:, ts], xbs[k][:, ts],
                                            m_bc[:, ts], op=OP.subtract)
                    nc.vector.scalar_tensor_tensor(
                        out=dst_tiles[k][:, ts], in0=t1[:, ts], scalar=g_col(k),
                        in1=r_bc[:, ts], op0=OP.mult, op1=OP.mult)
                    if not skip_bias:
                        nc.vector.tensor_scalar_add(
                            dst_tiles[k][:, ts], dst_tiles[k][:, ts], b_col(k))

    # ---------- initial residual (embedding+ll+wpe folded on host) ----------
    for k in range(KT):
        nc.sync.dma_start(h[k][:], d_h0[:, k * CH:(k + 1) * CH])

    # persistent weight-stream pool (bufs=1: layer l+1's loads wait until
    # layer l's qkv consumed them, which happens long before they're needed)
    wstream = ctx.enter_context(tc.tile_pool(name="wstream", bufs=1))
    mlpw = ctx.enter_context(tc.tile_pool(name="mlpw", bufs=1))
    # persistent attention operand tiles
    kvp = ctx.enter_context(tc.tile_pool(name="kvp", bufs=1))

    # ---------- transformer layers ----------
    for l in range(NL):
        # ln1
        y1 = [act.tile([128, CH], BF, tag=f"y{k}", name=f"y{k}") for k in range(KT)]
        layernorm(f"ln1_{l}", h, lambda k: PL("ln1g", l, k),
                  lambda k: PL("ln1b", l, k), flags["lnb0"], y1)

        with tc.tile_pool(name=f"qkt_{l}", bufs=1) as qp, \
             tc.tile_pool(name=f"qk_sb_{l}", bufs=3) as qsb:
            wqk = [wstream.tile([128, 1536], BF, tag=f"wqk{k}", name=f"wqk{k}") for k in range(KT)]
            wv = [wstream.tile([128, D], BF, tag=f"wv{k}", name=f"wv{k}") for k in range(KT)]
            for k in range(KT):
                nc.sync.dma_start(wqk[k][:], d_wqk[l, k * 128:(k + 1) * 128, :])
                nc.sync.dma_start(wv[k][:], d_wv[l, k * 128:(k + 1) * 128, :])
            # whole-layer fc weights in one DMA, after the qkv weights so it
            # never delays them; the 4.6MB transfer hides behind attention
            wfcall = mlpw.tile([128, FT * KT * 128], BF, tag="wfcall",
                               name=f"wfcall{l}")
            nc.sync.dma_start(wfcall[:], d_wfc[l])
            kTall = qp.tile([128, 6 * CH], BF, tag="kTall", name="kTall")
            kT = [kTall[:, m * CH:(m + 1) * CH] for m in range(6)]
            with tc.tile_pool(name=f"qk_ps_{l}", bufs=2, space="PSUM") as qpp:
                # k feature-major -> SBUF (own use) + k exchange buffer
                for m in range(6):
                    ps = qpp.tile([128, CH], F32, tag="qkps")
                    for k in range(KT):
                        nc.tensor.matmul(ps[:], wqk[k][:, 768 + m * 128:768 + (m + 1) * 128],
                                         y1[k][:], start=(k == 0), stop=(k == KT - 1))
                    if flags["bqk0"]:
                        nc.vector.tensor_copy(kT[m], ps[:])
                    else:
                        nc.vector.tensor_scalar_add(kT[m], ps[:], PL("bqk", l, 6 + m))
                    hf, mm = divmod(m, 3)
                    nc.gpsimd.dma_start(
                        d_kloc[l][hf][:, mm * 512:(mm + 1) * 512], kT[m])
                    # launch each half-exchange as soon as its tiles exist
                    if mm == 2:
                        nc.gpsimd.collective_compute(
                            "AllGather", mybir.AluOpType.bypass,
                            replica_groups=PAIRS,
                            ins=[d_kloc[l][hf][:]], outs=[d_kag[l][hf][:]])
                if True:
                    kpeer = kvp.tile([128, KT * 512], BF, tag="kpeer",
                                     name=f"kpeer{l}")
                    for hf in range(2):
                        nc.gpsimd.indirect_dma_start(
                            out=kpeer[:, hf * 1536:(hf + 1) * 1536],
                            out_offset=None, in_=d_kag[l][hf][:],
                            in_offset=bass.IndirectOffsetOnAxis(
                                ap=prow_sb[:, 0:1], axis=0))
                # v token-major -> v_tok own slots + v exchange buffer
                vball = qsb.tile([128, TO * 768], BF, tag="vball", bufs=1)
                for t in range(TO):
                    vps = qpp.tile([128, D], F32, tag="vps")
                    for nck, (noff, nsz) in enumerate([(0, 512), (512, 256)]):
                        for k in range(KT):
                            nc.tensor.matmul(
                                vps[:, noff:noff + nsz],
                                y1[k][:, t * 128:(t + 1) * 128],
                                wv[k][:, noff:noff + nsz],
                                start=(k == 0), stop=(k == KT - 1 and flags["bv0"]))
                        if not flags["bv0"]:
                            nc.tensor.matmul(vps[:, noff:noff + nsz],
                                             ones_row[:], bv_sb[l][:, noff:noff + nsz],
                                             start=False, stop=True)
                    nc.vector.tensor_copy(vball[:, t * 768:(t + 1) * 768],
                                          vps[:])
                    nc.gpsimd.dma_start(
                        d_vloc[l][:, t * 768:(t + 1) * 768],
                        vball[:, t * 768:(t + 1) * 768])
                    vt3 = v_tok[t][:].rearrange("p (h c) -> p h c", h=H)
                    vp3 = vps[:].rearrange("p (h c) -> p h c", h=H)
                    nc.vector.tensor_copy(vt3[:, :, 64:128], vp3[:, :, 0:64])
                # v exchange in flight while q is computed
                nc.gpsimd.collective_compute(
                    "AllGather", mybir.AluOpType.bypass, replica_groups=PAIRS,
                    ins=[d_vloc[l][:]], outs=[d_vag[l][:]])
                vpeer = kvp.tile([128, TO * 768], BF, tag="vpeer",
                                 name=f"vpeer{l}")
                nc.gpsimd.indirect_dma_start(
                    out=vpeer[:], out_offset=None, in_=d_vag[l][:],
                    in_offset=bass.IndirectOffsetOnAxis(ap=prow_sb[:, 0:1],
                                                        axis=0))
                # q feature-major into the zero-padded per-head operands
                for m in range(6):
                    ps = qpp.tile([128, CH], F32, tag="qkps")
                    for k in range(KT):
                        nc.tensor.matmul(ps[:], wqk[k][:, m * 128:(m + 1) * 128],
                                         y1[k][:], start=(k == 0), stop=(k == KT - 1))
                    if flags["bqk0"]:
                        nc.vector.tensor_copy(qpad[m][0:64, 0:512], ps[0:64, :])
                        nc.vector.tensor_copy(qpad[m][64:128, 512:1024], ps[64:128, :])
                    else:
                        o, _n = _PC["bqk"]
                        col = o + l * 12 + m
                        nc.vector.tensor_scalar_add(
                            qpad[m][0:64, 0:512], ps[0:64, :],
                            par[0:64, col:col + 1])
                        nc.vector.tensor_scalar_add(
                            qpad[m][64:128, 512:1024], ps[64:128, :],
                            par[64:128, col:col + 1])

            # attention: pass 1 covers own-chunk slots (no exchange dep) for
            # every head pair, pass 2 adds the peer-chunk slots once the
            # allgather lands, so the in-order tensor queue never stalls on it
            with tc.tile_pool(name=f"at_ot_{l}", bufs=1) as op_:
              oT = [op_.tile([128, CH], BF, tag=f"oT{k}", name=f"oT{k}") for k in range(KT)]
              with tc.tile_pool(name=f"at_sb_{l}", bufs=4) as ap_, \
                   tc.tile_pool(name=f"at_own_{l}", bufs=1) as aop, \
                   tc.tile_pool(name=f"at_ps_{l}", bufs=2, space="PSUM") as app:
                osb_own = {}
                for hpi in range(H // 2):
                    heads = (2 * hpi, 2 * hpi + 1)
                    opss = {heads[0]: app.tile([128, CH], F32, tag="ops0",
                                               name=f"ops0_{l}_{hpi}"),
                            heads[1]: app.tile([128, CH], F32, tag="ops1",
                                               name=f"ops1_{l}_{hpi}")}
                    for s in range(NOWN):
                        klhs = kTall[:, hpi * CH + s * 128:hpi * CH + (s + 1) * 128]
                        scm = app.tile([128, 2 * CH], F32, tag="scm",
                                       name="scm")
                        for hh in heads:
                            half = hh % 2
                            nc.tensor.matmul(
                                scm[:, half * 512:(half + 1) * 512], klhs,
                                qpad[hpi][:, half * 512:(half + 1) * 512],
                                start=True, stop=True)
                        exm = ap_.tile([128, 2 * CH], BF, tag="exm", bufs=3)
                        nc.scalar.activation(exm[:], scm[:], AF.Exp,
                                             bias=P("vkeep", s), scale=0.125)
                        nc.vector.tensor_tensor(exm[:], exm[:], trim[s][:],
                                                op=OP.mult)
                        for hh in heads:
                            half = hh % 2
                            nc.tensor.matmul(
                                opss[hh][:, :],
                                v_tok[s][:, hh * 128:hh * 128 + 128],
                                exm[:, half * 512:(half + 1) * 512],
                                start=(s == 0), stop=(s == NOWN - 1))
                    for hh in heads:
                        oso = aop.tile([128, CH], BF, tag=f"oso{hh}",
                                       name=f"oso{hh}_{l}")
                        nc.vector.tensor_copy(oso[:], opss[hh][:])  # frees psum
                        osb_own[hh] = oso
                # peer v columns: emitted here so pass-1 vector work is not
                # queued behind the exchange-dependent copies
                for i in range(NPEER):
                    vt3 = v_tok[TO + i][:].rearrange("p (h c) -> p h c", h=H)
                    vp3 = vpeer[:, i * 768:(i + 1) * 768].rearrange(
                        "p (h c) -> p h c", h=H)
                    nc.vector.tensor_copy(vt3[:, :, 64:128], vp3[:, :, 0:64])
                for hpi in range(H // 2):
                    heads = (2 * hpi, 2 * hpi + 1)
                    opsp = {}
                    if NPEER:
                        opsp = {heads[0]: app.tile([128, CH], F32, tag="ops0",
                                                   name=f"opsp0_{l}_{hpi}"),
                                heads[1]: app.tile([128, CH], F32, tag="ops1",
                                                   name=f"opsp1_{l}_{hpi}")}
                        for i in range(NPEER):
                            klhs = kpeer[:, hpi * 512 + i * 128:
                                         hpi * 512 + (i + 1) * 128]
                            scm = app.tile([128, 2 * CH], F32, tag="scm",
                                           name="scm")
                            for hh in heads:
                                half = hh % 2
                                nc.tensor.matmul(
                                    scm[:, half * 512:(half + 1) * 512], klhs,
                                    qpad[hpi][:, half * 512:(half + 1) * 512],
                                    start=True, stop=True)
                            exm = ap_.tile([128, 2 * CH], BF, tag="exm",
                                           bufs=3)
                            nc.scalar.activation(exm[:], scm[:], AF.Exp,
                                                 bias=P("pbias", i),
                                                 scale=0.125)
                            for hh in heads:
                                half = hh % 2
                                nc.tensor.matmul(
                                    opsp[hh][:, :],
                                    v_tok[TO + i][:, hh * 128:hh * 128 + 128],
                                    exm[:, half * 512:(half + 1) * 512],
                                    start=(i == 0),
                                    stop=(i == NPEER - 1))
                    for hh in heads:
                        qrow = slice((hh % 2) * 64, (hh % 2) * 64 + 64)
                        if NPEER:
                            osb = ap_.tile([128, CH], F32, tag="osb",
                                           name=f"osb{hh % 2}", bufs=3)
                            nc.vector.tensor_tensor(
                                osb[:], osb_own[hh][:], opsp[hh][:], op=OP.add)
                        else:
                            osb = osb_own[hh]
                        recf = ap_.tile([1, CH], F32, tag="recf",
                                        name=f"recf{hh % 2}", bufs=2)
                        nc.vector.reciprocal_approx_fast(out=recf[:],
                                                         in_=osb[0:1, :])
                        bcs = ap_.tile([128, CH], F32, tag="bcs", name=f"bcs{hh % 2}", bufs=2)
                        nc.gpsimd.partition_broadcast(bcs[:], recf[:])
                        nc.vector.tensor_tensor(
                            oT[hpi][qrow, :], osb[64:128, :], bcs[64:128, :], op=OP.mult)

              # wo + residual
              with tc.tile_pool(name=f"wo_ps_{l}", bufs=3, space="PSUM") as wop:
                    wo = [wstream.tile([128, D], BF, tag=f"wo{k}", name=f"wo{k}", bufs=1) for k in range(KT)]
                    for k in range(KT):
                        nc.sync.dma_start(wo[k][:], d_wo[l, k * 128:(k + 1) * 128, :])
                    for m in range(KT):
                        ps = wop.tile([128, CH], F32, tag="wops")
                        for k in range(KT):
                            nc.tensor.matmul(
                                ps[:], wo[k][:, m * 128:(m + 1) * 128],
                                oT[k][:], start=(k == 0), stop=(k == KT - 1))
                        nc.vector.scalar_tensor_tensor(
                            out=h[m][:], in0=ps[:], scalar=PL("bo", l, m),
                            in1=h[m][:], op0=OP.add, op1=OP.add)

        # ln2 + MLP
        y2 = [act.tile([128, CH], BF, tag=f"y{k}", name=f"y{k}") for k in range(KT)]
        layernorm(f"ln2_{l}", h, lambda k: PL("ln2g", l, k),
                  lambda k: PL("ln2b", l, k), flags["lnb0"], y2)
        with tc.tile_pool(name=f"mlp_sb_{l}", bufs=3) as mp, \
             tc.tile_pool(name=f"mlp_w_{l}", bufs=1) as mwp, \
             tc.tile_pool(name=f"mlp_ps_{l}", bufs=2, space="PSUM") as mpp:
            y3 = mwp.tile([128, FT * 512], BF, tag="y3")
            for m in range(FT):
                ps = mpp.tile([128, CH], F32, tag="fcps")
                for k in range(KT):
                    nc.tensor.matmul(
                        ps[:], wfcall[:, m * 768 + k * 128:m * 768 + (k + 1) * 128],
                        y2[k][:], start=(k == 0), stop=(k == KT - 1))
                if GELU_MODE == "hw":
                    nc.scalar.activation(y3[:, m * 512:(m + 1) * 512], ps[:],
                                         AF.Gelu_apprx_tanh,
                                         bias=PL("bfc", l, m), scale=1.0)
                else:
                    # x*sigmoid(1.702x) approximation (CoreSim-compatible)
                    assert flags["bfc0"], "sigmoid gelu path assumes zero bfc"
                    sg = mp.tile([128, CH], BF, tag="sg")
                    nc.scalar.activation(sg[:], ps[:], AF.Sigmoid, scale=1.702)
                    nc.vector.tensor_tensor(y3[:, m * 512:(m + 1) * 512],
                                            ps[:], sg[:], op=OP.mult)
            # pr: k-outer so wpr streams in quarter-layer chunks
            prps = [mpp.tile([128, CH], F32, tag=f"prps{mo}", bufs=1,
                             name=f"prps{mo}") for mo in range(KT)]
            for kc in range(4):
                wprc = mp.tile([128, 6 * 768], BF, tag="wprc", bufs=2)
                nc.sync.dma_start(
                    wprc[:], d_wpr[l][:, kc * 6 * 768:(kc + 1) * 6 * 768])
                for kk in range(6):
                    k = kc * 6 + kk
                    for mo in range(KT):
                        nc.tensor.matmul(
                            prps[mo][:],
                            wprc[:, kk * 768 + mo * 128:kk * 768 + (mo + 1) * 128],
                            y3[:, k * 512:(k + 1) * 512],
                            start=(k == 0), stop=(k == FT - 1))
            for mo in range(KT):
                ps = prps[mo]
                nc.vector.scalar_tensor_tensor(
                    out=h[mo][:], in0=ps[:], scalar=PL("bpr", l, mo),
                    in1=h[mo][:], op0=OP.add, op1=OP.add)

    # ---------- final LN + LM head ----------
    with tc.tile_pool(name="lm_w", bufs=3) as lwp, \
         tc.tile_pool(name="lm_o", bufs=6) as lop, \
         tc.tile_pool(name="lm_ps", bufs=4, space="PSUM") as lpp:
        # prefetch the first weight tiles while lnf runs
        pre_w = []
        for vt in range(2):
            w = lwp.tile([128, KT * 512], BF, tag="lmw")
            nc.sync.dma_start(w[:], d_lmw[vt])
            pre_w.append(w)
        yf = [act.tile([128, CH], BF, tag=f"y{k}", name=f"y{k}") for k in range(KT)]
        layernorm("lnf", h, lambda k: P("lnfg", k), lambda k: P("lnfb", k),
                  flags["lnb0"], yf)
        for vt in range(NVT):
            if vt < 2:
                w = pre_w[vt]
            else:
                w = lwp.tile([128, KT * 512], BF, tag="lmw")
                nc.sync.dma_start(w[:], d_lmw[vt])
            for t in range(TO):
                ps = lpp.tile([128, 512], F32, tag="lmps")
                for k in range(KT):
                    nc.tensor.matmul(ps[:], yf[k][:, t * 128:(t + 1) * 128],
                                     w[:, k * 512:(k + 1) * 512],
                                     start=(k == 0), stop=(k == KT - 1))
                ob = lop.tile([128, 512], BF, tag="ob")
                nc.vector.tensor_copy(ob[:], ps[:])
                nc.scalar.dma_start(
                    d_out[t * 128:(t + 1) * 128, vt * 512:(vt + 1) * 512], ob[:])


def build(flags):
    nc = bacc.Bacc("TRN2", target_bir_lowering=False, debug=False, num_devices=8)
    with tile.TileContext(nc) as tc, ExitStack() as ctx:
        nc._tc = tc
        nc._ctx = ctx
        _emit(nc, flags)
    nc.compile()
    return nc


def host_prep(inputs):
    """Returns (in_maps for 8 cores, flags)."""
    bf16 = ml_dtypes.bfloat16
    g = {k: np.asarray(v) for k, v in inputs.items()}

    tok = np.zeros((B, L), np.int32)
    valid = np.zeros((B, L), np.float32)
    for b in range(B):
        seq = np.concatenate([
            g["ctx"][b, :int(g["c_lens"][b])],
            g["c2"][b, :int(g["c2_lens"][b])],
            g["query"][b, :int(g["q_lens"][b])],
            g["response"][b, :int(g["r_lens"][b])]]).astype(np.int32)
        tok[b, :len(seq)] = seq
        valid[b, :len(seq)] = 1.0

    wte = g["wte"].astype(np.float32)
    wte_bf = wte.astype(bf16)
    # lm head tiles, full padded vocab: [NVT, 128, KT*512]
    wh = np.zeros((VPAD, D), np.float32)
    wh[:V] = wte
    a = wh.reshape(NVT, 512, KT, 128).transpose(0, 3, 2, 1)  # [vt, p, k, n]
    lmw = np.ascontiguousarray(a.reshape(NVT, 128, KT * 512)).astype(bf16)

    llw_bf = g["ll_w"].astype(np.float32).astype(bf16)  # bf16-rounded like device
    wqkv = g["wqkv"].astype(np.float32)
    wqk = np.ascontiguousarray(wqkv[:, :, :1536]).astype(bf16)
    wv = np.ascontiguousarray(wqkv[:, :, 1536:]).astype(bf16)
    bv = np.ascontiguousarray(g["bqkv"][:, 1536:]).astype(np.float32).astype(bf16)
    wo = g["wo"].astype(np.float32).astype(bf16)
    # fc: [128, f*768 + k*128 + n] = wfc[k*128+p, f*128+n]
    wfc_t = np.zeros((NL, 128, FT * KT * 128), np.float32)
    for l in range(NL):
        a = g["wfc"][l].astype(np.float32).reshape(KT, 128, FT, 128)
        wfc_t[l] = a.transpose(1, 2, 0, 3).reshape(128, FT * KT * 128)
    wfc_t = wfc_t.astype(bf16)
    # pr: [128, k*768 + mo*128 + n] = wpr[k*128+p, mo*128+n]
    wpr_t = np.zeros((NL, 128, FT * KT * 128), np.float32)
    for l in range(NL):
        a = g["wpr"][l].astype(np.float32).reshape(FT, 128, KT, 128)
        wpr_t[l] = a.transpose(1, 0, 2, 3).reshape(128, FT * KT * 128)
    wpr = wpr_t.astype(bf16)

    def pp(x, nt):  # [nt*128] -> [128, nt] col-per-tile
        return np.ascontiguousarray(np.asarray(x, np.float32).reshape(nt, 128).T)

    par_base = np.zeros((128, PCOLS), np.float32)
    def setp(name, arr):
        o, n = _PC[name]
        par_base[:, o:o + n] = arr
    setp("llb", pp(g["ll_b"], KT))
    setp("lnfg", pp(g["lnf_g"], KT))
    setp("lnfb", pp(g["lnf_b"], KT))
    setp("ln1g", np.concatenate([pp(g["ln1_g"][l], KT) for l in range(NL)], 1))
    setp("ln1b", np.concatenate([pp(g["ln1_b"][l], KT) for l in range(NL)], 1))
    setp("ln2g", np.concatenate([pp(g["ln2_g"][l], KT) for l in range(NL)], 1))
    setp("ln2b", np.concatenate([pp(g["ln2_b"][l], KT) for l in range(NL)], 1))
    setp("bqk", np.concatenate([pp(g["bqkv"][l, :1536], 12) for l in range(NL)], 1))
    setp("bo", np.concatenate([pp(g["bo"][l], KT) for l in range(NL)], 1))
    setp("bfc", np.concatenate([pp(g["bfc"][l], FT) for l in range(NL)], 1))
    setp("bpr", np.concatenate([pp(g["bpr"][l], KT) for l in range(NL)], 1))

    totals = (np.asarray(g["c_lens"]) + np.asarray(g["c2_lens"])
              + np.asarray(g["q_lens"]) + np.asarray(g["r_lens"]))
    flags = {
        "maxkt": int(np.ceil(int(totals.max()) / 128)),
        "bqk0": not np.any(g["bqkv"][:, :1536]),
        "bv0": not np.any(g["bqkv"][:, 1536:]),
        "lnb0": not (np.any(g["ln1_b"]) or np.any(g["ln2_b"]) or np.any(g["lnf_b"])),
        "bfc0": not np.any(g["bfc"]),
    }

    shared = dict(wqk=wqk, wv=wv, bv=bv,
                  wo=wo, wfc=wfc_t, wpr=wpr, lmw=lmw)
    p_idx = np.arange(128)
    in_maps = []
    for core in range(8):
        b, c = core // 2, core % 2
        total_b = int(totals[b])
        m = dict(shared)
        toks = tok[b, c * 512:(c + 1) * 512]
        vmask = valid[b, c * 512:(c + 1) * 512]
        embg = (wte[toks] * vmask[:, None]).astype(bf16).astype(np.float32)
        h0 = (embg @ llw_bf.T.astype(np.float32) + g["ll_b"].astype(np.float32)
              + g["wpe"].astype(np.float32)[c * 512:(c + 1) * 512])
        # feature-major: [p_feat(128), k*CH + tok]
        m["h0"] = np.ascontiguousarray(
            h0.T.reshape(KT, 128, CH).transpose(1, 0, 2).reshape(128, KT * CH)
        ).astype(np.float32)
        # peer half of the pair allgather buffer: rank (1-c) rows
        m["prow"] = np.ascontiguousarray(
            ((1 - c) * 128 + p_idx).reshape(128, 1).astype(np.int32))
        par = par_base.copy()
        o, n = _PC["valid"]
        par[:, o:o + n] = valid[b, c * 512:(c + 1) * 512].reshape(TO, 128).T
        # own-slot exp bias: key validity only (causality via static tri mask)
        o, n = _PC["vkeep"]
        own_key = 512 * c + p_idx[:, None] + 128 * np.arange(TO)[None, :]
        par[:, o:o + n] = np.where(own_key < total_b, 0.0, NEGBIG)
        # peer-slot exp bias: chunk-1 cores keep valid chunk-0 keys;
        # chunk-0 cores mask the whole peer chunk (acausal)
        o, n = _PC["pbias"]
        peer_key = 512 * (1 - c) + p_idx[:, None] + 128 * np.arange(TO)[None, :]
        if c == 1:
            par[:, o:o + n] = np.where(peer_key < total_b, 0.0, NEGBIG)
        else:
            par[:, o:o + n] = NEGBIG
        m["par"] = par
        in_maps.append(m)
    return in_maps, flags


def _assemble(results):
    outs = []
    for b in range(B):
        o0 = results[2 * b]["out"][:, :V]
        o1 = results[2 * b + 1]["out"][:, :V]
        outs.append(np.concatenate([o0, o1], axis=0))
    return np.stack(outs).astype(np.float32)


def kernel(**inputs):
    in_maps, flags = host_prep(inputs)
    nc = build(flags)
    res = run_bass_kernel_spmd(nc, in_maps, list(range(8)))
    return _assemble(res.results)


def _install_profile_shims():
    """This container's antenv lacks axon_hooks; rebuild the NTFF hook from
    trn_agent_boot's ctypes helper and stub out the S3 artifact upload."""
    import sys, types
    try:
        import antenv.axon_hooks  # noqa: F401
    except ImportError:
        from trn_agent_boot.trn_boot import _ntff_profile_via_ctypes
        hook = _ntff_profile_via_ctypes("/opt/axon/libaxon_pjrt.so")
        m = types.ModuleType("antenv.axon_hooks")
        m.get_axon_ntff_profile_hook = lambda: hook
        m.set_axon_ntff_profile_hook = lambda h: None
        sys.modules["antenv.axon_hooks"] = m
        import antenv
        antenv.axon_hooks = m
    import concourse.bass_utils as bu
    bu.upload_artifacts = lambda tmpdir: tmpdir


def kernel_traced(tmpdir=None, **inputs):
    """Like kernel() but returns (output, exec_time_ns)."""
    _install_profile_shims()
    in_maps, flags = host_prep(inputs)
    nc = build(flags)
    res = run_bass_kernel_spmd(nc, in_maps, list(range(8)), trace=True,
                               tmpdir=tmpdir)
    return _assemble(res.results), res.exec_time_ns
